# revision 1
# baseline (speedup 1.0000x reference)
"""Trainium2 Bass kernel for nn_Attention_85710367359111.

Full (unsharded) numpy inputs in, full output out. Internally:
tensor-parallel over heads (16 heads / 8 cores = 2 heads per core),
on-device AllToAll to re-shard from head-parallel to token-parallel
before the output projection, host-side concat of the 8 token blocks.

Per-core pipeline (all feature-major [feat_on_partitions, tokens]):
  A  qkv^T = w_qkv_slice @ x^T           (PE, fp32r)
  B  RMS stats via selector-matmul -> sqrt(ACT) -> reciprocal(DVE),
     scale+norm_w via rank-2 broadcast matmul, RoPE via +-1 permutation
     matmul + cos/sin elementwise (DVE)
  B' v^T -> v token-major via PE transposes (with appended ones column)
  C  scores^T = k^T.T @ q^T per (batch,head) -> exp (ACT) ->
     o_aug^T = [v|1].T @ exp(scores^T)  (PSUM-accumulated over k tiles),
     softmax denominator = last row of o_aug; normalize via
     reciprocal + rank-1 broadcast matmul + multiply
  D  DMA o to DRAM laid out [tok_block, d_local, tok_in], AllToAll,
     out^T = w_out^T.T @ gathered  (row-parallel, no all-reduce needed)
"""

import math
import os

import numpy as np

import concourse.bacc as bacc
import concourse.bass as bass
import concourse.tile as tile
from concourse import mybir
from concourse.bass_utils import run_bass_kernel_spmd

# ---------------------------------------------------------------- config

B, S, D, H, E = 2, 2048, 1024, 16, 64
NC = 8                      # cores
HPC = H // NC               # heads per core = 2
DL = HPC * E                # local d slice = 128
T = B * S                   # total tokens
TB = T // NC                # tokens per A2A block
KT = S // 128               # k tiles per batch
NTT = T // 128              # total tok tiles
QC = min(512, S)            # q chunk
NQC = S // QC               # q chunks per batch
EPS = float(np.finfo(np.float32).eps)

MM_DT = mybir.dt.float32r if os.environ.get("KMM_DT", "f32r") == "f32r" else mybir.dt.float32


def _r(ap):
    return ap


def build_nc(s=S, reps=1):
    global S, T, TB, KT, NTT, QC, NQC
    S = s
    T = B * S
    TB = T // NC
    KT = S // 128
    NTT = T // 128
    QC = min(512, S)
    NQC = S // QC

    f32 = mybir.dt.float32
    nc = bacc.Bacc("TRN2", target_bir_lowering=False, debug=False, num_devices=NC)

    # ------------- DRAM I/O
    rdt = MM_DT
    xT_d = nc.dram_tensor("xT", [D, T], rdt, kind="ExternalInput")
    wq_d = nc.dram_tensor("wqT", [D, DL], rdt, kind="ExternalInput")
    wk_d = nc.dram_tensor("wkT", [D, DL], rdt, kind="ExternalInput")
    wv_d = nc.dram_tensor("wvT", [D, DL], rdt, kind="ExternalInput")
    wo_d = nc.dram_tensor("woT", [D, D], rdt, kind="ExternalInput")
    cos_d = nc.dram_tensor("cosT", [128, T], f32, kind="ExternalInput")
    sin_d = nc.dram_tensor("sinT", [128, T], f32, kind="ExternalInput")
    sel_d = nc.dram_tensor("sel", [128, 2], f32, kind="ExternalInput")
    wsel_d = nc.dram_tensor("wsel", [2, 128], f32, kind="ExternalInput")
    perm_d = nc.dram_tensor("permT", [128, 128], f32, kind="ExternalInput")
    iden_d = nc.dram_tensor("iden", [128, 128], f32, kind="ExternalInput")
    ones_d = nc.dram_tensor("ones1", [1, E], f32, kind="ExternalInput")
    epsb_d = nc.dram_tensor("epsb", [2, 1], f32, kind="ExternalInput")
    onec_d = nc.dram_tensor("onecol", [128, NTT], rdt, kind="ExternalInput")

    osh_d = nc.dram_tensor("o_shard", [NC * DL, TB], rdt)
    oga_d = nc.dram_tensor("o_gath", [NC * DL, TB], rdt)
    out_d = nc.dram_tensor("out_t", [D, TB], f32, kind="ExternalOutput")
    DBG = bool(os.environ.get("KDEBUG"))
    PH = os.environ.get("KPHASES", "ABCD")
    if DBG:
        dbg = {}
        for nm, shp, dt_ in [("dbg_q01", [128, T], f32), ("dbg_k01", [128, T], f32),
                             ("dbg_v01", [128, T], f32), ("dbg_qhat", [128, T], rdt),
                             ("dbg_khat", [128, T], rdt), ("dbg_vtok", [128, NTT * 130], rdt),
                             ("dbg_osh", [NC * DL, TB], rdt), ("dbg_oga", [NC * DL, TB], rdt),
                             ("dbg_ex", [128, 2 * QC], rdt), ("dbg_osb", [65, QC], f32)]:
            dbg[nm] = nc.dram_tensor(nm, shp, dt_, kind="ExternalOutput")

    xT_v = xT_d.ap().rearrange("(dt p) t -> dt p t", p=128)     # [8,128,T]
    wo_v = wo_d.ap().rearrange("(dt p) o -> dt p o", p=128)     # [8,128,D]
    oga_v = oga_d.ap().rearrange("(dt p) t -> dt p t", p=128)   # [8,128,TB]

    from contextlib import ExitStack

    with tile.TileContext(nc) as tc, ExitStack() as ctx:
      for _rep in range(reps):
          with ExitStack() as ctx2:
            # persistent (whole-kernel) buffers
            pers = ctx2.enter_context(tc.tile_pool(name="pers", bufs=1))
            qhat = pers.tile([128, T], rdt, tag="qhat")     # [q_h0; q_h1] normed+roped
            khat = pers.tile([128, T], rdt, tag="khat")
            vtok = pers.tile([128, NTT, 130], rdt, tag="vtok")  # token-major v + ones cols
            wq_s = pers.tile([128, D], rdt, tag="wq")       # w tiles, [128(dt rows), 8*128]
            wk_s = pers.tile([128, D], rdt, tag="wk")
            wv_s = pers.tile([128, D], rdt, tag="wv")
            sel_s = pers.tile([128, 2], f32, tag="sel")
            wsel_s = pers.tile([2, 128], f32, tag="wsel")
            perm_s = pers.tile([128, 128], f32, tag="perm")
            iden_s = pers.tile([128, 128], f32, tag="iden")
            ones_s = pers.tile([1, E], f32, tag="ones1")
            epsb_s = pers.tile([2, 1], f32, tag="epsb")

            for dt_i in range(8):
                nc.sync.dma_start(wq_s[:, bass.ts(dt_i, 128)], wq_d.ap()[bass.ts(dt_i, 128), :])
                nc.sync.dma_start(wk_s[:, bass.ts(dt_i, 128)], wk_d.ap()[bass.ts(dt_i, 128), :])
                nc.sync.dma_start(wv_s[:, bass.ts(dt_i, 128)], wv_d.ap()[bass.ts(dt_i, 128), :])
            nc.sync.dma_start(sel_s[:], sel_d.ap())
            nc.sync.dma_start(wsel_s[:], wsel_d.ap())
            nc.sync.dma_start(perm_s[:], perm_d.ap())
            nc.sync.dma_start(iden_s[:], iden_d.ap())
            nc.sync.dma_start(ones_s[:], ones_d.ap())
            nc.sync.dma_start(epsb_s[:], epsb_d.ap())

            # ones columns of vtok (col 64 for head0, col 129 for head1)
            nc.sync.dma_start(vtok[:, :, 64], onec_d.ap())
            nc.sync.dma_start(vtok[:, :, 129], onec_d.ap())

            NCH = T // 512  # 512-wide token chunks for phases A/B

            with tc.tile_pool(name="ab", bufs=1) as ab, \
                 tc.tile_pool(name="xt", bufs=12) as xtp, \
                 tc.tile_pool(name="sq", bufs=3) as sqp, \
                 tc.tile_pool(name="cs", bufs=3) as csp, \
                 tc.tile_pool(name="st", bufs=3) as stp, \
                 tc.tile_pool(name="ps", bufs=2, space=bass.MemorySpace.PSUM) as psp, \
                 tc.tile_pool(name="pss", bufs=2, space=bass.MemorySpace.PSUM) as pss, \
                 tc.tile_pool(name="tmp", bufs=3) as tmpp:
                q01 = ab.tile([128, T], f32, tag="q01")
                k01 = ab.tile([128, T], f32, tag="k01")
                v01 = ab.tile([128, T], f32, tag="v01")

                # ---------------- phase A: qkv^T = W @ x^T, feature-major
                for c in range(NCH):
                    cs = bass.ts(c, 512)
                    xts = []
                    for dt_i in range(8):
                        xt = xtp.tile([128, 512], rdt, tag="xt")
                        nc.sync.dma_start(xt[:], xT_v[dt_i, :, cs])
                        xts.append(xt)
                    for w_s, dest in ((wq_s, q01), (wk_s, k01), (wv_s, v01)):
                        ps = psp.tile([128, 512], f32, tag="ps")
                        for dt_i in range(8):
                            nc.tensor.matmul(
                                ps[:], _r(w_s[:, bass.ts(dt_i, 128)]), _r(xts[dt_i][:]),
                                start=(dt_i == 0), stop=(dt_i == 7),
                            )
                        nc.scalar.copy(dest[:, cs], ps[:])

                # ------- phase B (fused): stats -> alpha -> scale -> rope, per chunk
                for c in range(NCH if "B" in PH else 0):
                    cs = bass.ts(c, 512)
                    cos_t = csp.tile([128, 512], f32, tag="cos")
                    sin_t = csp.tile([128, 512], f32, tag="sin")
                    nc.sync.dma_start(cos_t[:], cos_d.ap()[:, cs])
                    nc.sync.dma_start(sin_t[:], sin_d.ap()[:, cs])
                    for src_t, dest in ((q01, qhat), (k01, khat)):
                        sq = sqp.tile([128, 512], f32, tag="sq")
                        nc.scalar.activation(sq[:], src_t[:, cs],
                                             mybir.ActivationFunctionType.Square)
                        st = pss.tile([2, 512], f32, tag="pss")
                        nc.tensor.matmul(st[:], _r(sel_s[:]), _r(sq[:]), start=True, stop=True)
                        sqv = stp.tile([2, 512], f32, tag="sqv")
                        # sqrt(8*mean + 8*eps); reciprocal gives alpha/sqrt(8)
                        nc.scalar.activation(sqv[:], st[:],
                                             mybir.ActivationFunctionType.Sqrt,
                                             bias=epsb_s[:], scale=8.0)
                        alpha = stp.tile([2, 512], f32, tag="alpha")
                        nc.vector.reciprocal_approx_fast(alpha[:], sqv[:])
                        sps = pss.tile([128, 512], f32, tag="pss")
                        nc.tensor.matmul(sps[:], _r(wsel_s[:]), _r(alpha[:]),
                                         start=True, stop=True)
                        qs = tmpp.tile([128, 512], f32, tag="qs")
                        nc.vector.tensor_mul(qs[:], src_t[:, cs], sps[:])
                        yp = psp.tile([128, 512], f32, tag="ps")
                        nc.tensor.matmul(yp[:], _r(perm_s[:]), _r(qs[:]), start=True, stop=True)
                        t1 = tmpp.tile([128, 512], f32, tag="t1")
                        nc.vector.tensor_mul(t1[:], qs[:], cos_t[:])
                        t2 = tmpp.tile([128, 512], f32, tag="t2")
                        nc.vector.tensor_mul(t2[:], yp[:], sin_t[:])
                        nc.vector.tensor_add(dest[:, cs], t1[:], t2[:])

                if DBG:
                    nc.sync.dma_start(dbg["dbg_q01"].ap(), q01[:])
                    nc.sync.dma_start(dbg["dbg_k01"].ap(), k01[:])
                    nc.sync.dma_start(dbg["dbg_v01"].ap(), v01[:])
                    nc.sync.dma_start(dbg["dbg_qhat"].ap(), qhat[:])
                    nc.sync.dma_start(dbg["dbg_khat"].ap(), khat[:])

                # ---------------- phase B': v -> token-major (+ ones)
                for g in range(NTT // 4 if "B" in PH else 0):
                    pt = psp.tile([128, 4, 128], f32, tag="ps")
                    for j in range(4):
                        tt = g * 4 + j
                        nc.tensor.transpose(pt[:, j, :], v01[:, bass.ts(tt, 128)], iden_s[:])
                    nc.vector.tensor_copy(vtok[:, bass.ts(g, 4), 0:64], pt[:, :, 0:64])
                    nc.vector.tensor_copy(vtok[:, bass.ts(g, 4), 65:129], pt[:, :, 64:128])

            # ---------------- phase C: attention per (batch, qchunk)
            wop = ctx2.enter_context(tc.tile_pool(name="wo", bufs=1))
            wo_s = wop.tile([128, 8, D], rdt, tag="wo")
            for dt_i in range(8):
                nc.sync.dma_start(wo_s[:, dt_i, :], wo_v[dt_i])
            with tc.tile_pool(name="scps", bufs=2, space=bass.MemorySpace.PSUM) as scps, \
                 tc.tile_pool(name="ops", bufs=1, space=bass.MemorySpace.PSUM) as ops, \
                 tc.tile_pool(name="bcps", bufs=1, space=bass.MemorySpace.PSUM) as bcps, \
                 tc.tile_pool(name="expp", bufs=4) as expp, \
                 tc.tile_pool(name="osb", bufs=2) as osbp, \
                 tc.tile_pool(name="den", bufs=2) as denp, \
                 tc.tile_pool(name="ofin", bufs=2) as ofinp:
                for b in range(B if "C" in PH else 0):
                    for qc in range(NQC):
                        qs_ = slice(b * S + qc * QC, b * S + (qc + 1) * QC)
                        oa0 = ops.tile([65, QC], f32, tag="oa0")
                        oa1 = ops.tile([65, QC], f32, tag="oa1")
                        for kt in range(KT):
                            ks = slice(b * S + kt * 128, b * S + (kt + 1) * 128)
                            sc = scps.tile([128, 2 * QC], f32, tag="sc")
                            nc.tensor.matmul(sc[:, 0:QC], _r(khat[0:64, ks]),
                                             _r(qhat[0:64, qs_]), start=True, stop=True)
                            nc.tensor.matmul(sc[:, QC:2 * QC], _r(khat[64:128, ks]),
                                             _r(qhat[64:128, qs_]), start=True, stop=True)
                            ex = expp.tile([128, 2 * QC], rdt, tag="ex")
                            nc.scalar.activation(ex[:], sc[:], mybir.ActivationFunctionType.Exp)
                            if DBG and b == 0 and qc == 0 and kt == 0:
                                nc.sync.dma_start(dbg["dbg_ex"].ap(), ex[:])
                            tt = b * KT + kt
                            for h in range(HPC):
                                nc.tensor.matmul(
                                    (oa0 if h == 0 else oa1)[:],
                                    _r(vtok[:, tt, h * 65:h * 65 + 65]),
                                    _r(ex[:, bass.ts(h, QC)]),
                                    start=(kt == 0), stop=(kt == KT - 1),
                                )
                        for h, oa in ((0, oa0), (1, oa1)):
                            osb = osbp.tile([65, QC], f32, tag="osb")
                            nc.vector.tensor_copy(osb[:], oa[:])
                            if DBG and b == 0 and qc == 0 and h == 0:
                                nc.sync.dma_start(dbg["dbg_osb"].ap(), osb[:])
                            den0 = denp.tile([1, QC], f32, tag="den0")
                            nc.sync.dma_start(den0[:], osb[64:65, :])
                            dr = denp.tile([1, QC], f32, tag="dr")
                            nc.vector.reciprocal_approx_fast(dr[:], den0[:])
                            bc = bcps.tile([64, QC], f32, tag="bc")
                            nc.tensor.matmul(bc[:], _r(ones_s[:]), _r(dr[:]),
                                             start=True, stop=True)
                            of = ofinp.tile([64, QC], rdt, tag="of")
                            nc.vector.tensor_mul(of[:], osb[0:64, :], bc[:])
                            # o_shard row = tok_block*DL + h*64
                            tok0 = b * S + qc * QC
                            if QC <= TB:
                                blk = tok0 // TB
                                off = tok0 % TB
                                nc.sync.dma_start(
                                    osh_d.ap()[blk * DL + h * 64: blk * DL + h * 64 + 64,
                                               off:off + QC], of[:])
                            else:
                                for sb in range(QC // TB):
                                    blk = (tok0 + sb * TB) // TB
                                    nc.sync.dma_start(
                                        osh_d.ap()[blk * DL + h * 64: blk * DL + h * 64 + 64, :],
                                        of[:, bass.ts(sb, TB)])

                # ---------------- phase D: AllToAll + out projection
                if DBG:
                    nc.sync.dma_start(dbg["dbg_vtok"].ap(),
                                      vtok[:].rearrange("p a b -> p (a b)"))
                    nc.sync.dma_start(dbg["dbg_osh"].ap(), osh_d.ap())
                if not os.environ.get("KNO_CC"):
                    nc.gpsimd.collective_compute(
                        "AllToAll", mybir.AluOpType.bypass,
                        replica_groups=[list(range(NC))],
                        ins=[osh_d.ap()], outs=[oga_d.ap()],
                    )

            with tc.tile_pool(name="gd", bufs=1) as gdp, \
                 tc.tile_pool(name="pso", bufs=2, space=bass.MemorySpace.PSUM) as psop, \
                 tc.tile_pool(name="osb2", bufs=2) as osb2p:
                g_s = gdp.tile([128, 8, TB], rdt, tag="g")
                ga_v = osh_d.ap().rearrange("(dt p) t -> dt p t", p=128) \
                    if os.environ.get("KNO_CC") else oga_v
                for dt_i in range(8):
                    nc.sync.dma_start(g_s[:, dt_i, :], ga_v[dt_i])
                if DBG and not os.environ.get("KNO_CC"):
                    nc.sync.dma_start(dbg["dbg_oga"].ap(), oga_d.ap())
                for do in range(8 if "D" in PH else 0):
                    po = psop.tile([128, TB], f32, tag="pso")
                    for dt_i in range(8):
                        nc.tensor.matmul(po[:], _r(wo_s[:, dt_i, bass.ts(do, 128)]),
                                         _r(g_s[:, dt_i, :]),
                                         start=(dt_i == 0), stop=(dt_i == 7))
                    ob = osb2p.tile([128, TB], f32, tag="ob")
                    nc.scalar.copy(ob[:], po[:])
                    nc.sync.dma_start(out_d.ap()[bass.ts(do, 128), :], ob[:])

    nc.compile()
    return nc


def make_inputs(x, position, w_qkv, w_out, norm_w, s=None):
    """Build the 8 per-core input dicts from full inputs."""
    s = s or x.shape[1]
    t = x.shape[0] * s
    xT = np.ascontiguousarray(x.reshape(t, D).T).astype(np.float32)
    cos = position[0]   # [s, E]
    sin = position[1]
    cosT1 = np.ascontiguousarray(cos.T)          # [E, s]
    sinT1 = np.ascontiguousarray(sin.T)
    cosT = np.tile(np.concatenate([cosT1, cosT1], 0), (1, x.shape[0]))  # [128, t]
    sinT = np.tile(np.concatenate([sinT1, sinT1], 0), (1, x.shape[0]))

    sel = np.zeros((128, 2), np.float32)
    sel[0:64, 0] = 1.0 / 64.0
    sel[64:128, 1] = 1.0 / 64.0
    wsel = np.zeros((2, 128), np.float32)
    wsel[0, 0:64] = norm_w
    wsel[1, 64:128] = norm_w
    # rope: y = P t ;  y[i] = -t[2i+1] (i<32), y[32+i] = t[2i]
    P = np.zeros((64, 64), np.float32)
    for i in range(32):
        P[i, 2 * i + 1] = -1.0
        P[32 + i, 2 * i] = 1.0
    Pb = np.zeros((128, 128), np.float32)
    Pb[0:64, 0:64] = P
    Pb[64:128, 64:128] = P
    permT = np.ascontiguousarray(Pb.T)
    iden = np.eye(128, dtype=np.float32)
    ones1 = np.ones((1, E), np.float32)
    woT = np.ascontiguousarray(w_out.T).astype(np.float32)

    w3 = w_qkv.reshape(H, 3, E, D)
    in_maps = []
    for c in range(NC):
        h0, h1 = HPC * c, HPC * c + 1
        wqT = np.ascontiguousarray(
            np.concatenate([w3[h0, 0], w3[h1, 0]], 0).T).astype(np.float32)
        wkT = np.ascontiguousarray(
            np.concatenate([w3[h0, 1], w3[h1, 1]], 0).T).astype(np.float32)
        wvT = np.ascontiguousarray(
            np.concatenate([w3[h0, 2], w3[h1, 2]], 0).T).astype(np.float32)
        in_maps.append({
            "xT": xT, "wqT": wqT, "wkT": wkT, "wvT": wvT, "woT": woT,
            "cosT": cosT.astype(np.float32), "sinT": sinT.astype(np.float32),
            "sel": sel, "wsel": wsel, "permT": permT, "iden": iden, "ones1": ones1,
            "epsb": np.full((2, 1), 8.0 * EPS, np.float32),
            "onecol": np.ones((128, t // 128), np.float32),
        })
    return in_maps


def assemble(results, s=None):
    s = s or S
    t = B * s
    tb = t // NC
    out = np.empty((t, D), np.float32)
    for c in range(NC):
        out[c * tb:(c + 1) * tb, :] = results[c]["out_t"].T
    return out.reshape(B, s, D)


_NC_CACHE = {}


def kernel(x, position, w_qkv, w_out, norm_w, heads):
    x = np.asarray(x, np.float32)
    position = np.asarray(position, np.float32)
    w_qkv = np.asarray(w_qkv, np.float32)
    w_out = np.asarray(w_out, np.float32)
    norm_w = np.asarray(norm_w, np.float32)
    s = x.shape[1]
    if s not in _NC_CACHE:
        _NC_CACHE[s] = build_nc(s)
    nc = _NC_CACHE[s]
    in_maps = make_inputs(x, position, w_qkv, w_out, norm_w, s=s)
    res = run_bass_kernel_spmd(nc, in_maps, list(range(NC)))
    return assemble(res.results, s=s)

